# revision 9
# baseline (speedup 1.0000x reference)
"""CTC loss kernel for Trainium2 (8 NeuronCores, data-parallel over batch).

Strategy
--------
Per core: 64 examples. The CTC forward DP runs in probability space
(4 tensor ops per time step on DVE) with states in the free dim and
(example, direction) packed into the 128 partitions: rows 0-63 run the
forward DP for t=0..255, rows 64-127 run the suffix (backward) DP in
state-reversed coordinates for t=511..256.  The two halves are spliced
at T/2:  P = sum_s alpha_255[s] * W_255[s].

Emissions E[b,t,s] = g*(y_pred[b,t,ext_b[s]] + eps) are gathered on the
GPSIMD engine (ap_gather, per-example class gather with t in the
partition dim), cast+eps+boost fused into one scalar-engine activation,
then reshuffled to (example-partition, t-major) layout via a DRAM
round-trip (both DMA directions fully strided-contiguous).

Numerics: bf16 DP state, per-8-step per-example rescale to a 2^43
setpoint (max history written out, logs added back on host), pad states
gather a zeroed 97th class so they decay.  The final splice
P = sum_s alpha[s]*W[s] spans ~e^-180 for tail examples, far outside
f32 range, so the two final state tiles (34KB each) are DMA'd out and
the splice runs on host in f64 (the per-example log + mean were host
work already).
"""

import numpy as np

B, T, C, L = 512, 512, 96, 64
BLANK = C - 1
EPS = 1e-7
S = 2 * L + 1          # 129 states
SW = 132               # padded state width (multiple of 4)
NIDX = 144             # gather width (multiple of 16)
CW = C + 1             # classes + one zeroed column (pad target)
NCORES = 8
BN = B // NCORES       # 64 examples per core
TH = T // 2            # split point
RESC = 8               # rescale period
NRESC = (TH - 1) // RESC  # 31 rescales (tau = 8,16,...,248)
SETPOINT_LOG2 = 43     # rescale normalizes row max to 2^43 (centers bf16 range)
G = 60.646622          # exp(mean_loss/T) boost; keeps alpha ~O(1) per step

_BUILT = None
_LAST_EXEC_NS = None
_LAST_RES = None


def _host_metadata(y_true):
    """ext labels, skip masks, init masks, gather indices — tiny, from y_true."""
    y_true = np.asarray(y_true, dtype=np.int32)
    lbl_len = (y_true != -1).sum(axis=-1).astype(np.int32)
    labels = np.where(y_true != -1, y_true, 0).astype(np.int32)
    ext = np.full((B, S), BLANK, np.int32)
    ext[:, 1::2] = labels
    ext_m2 = np.pad(ext[:, :-2], ((0, 0), (2, 0)), constant_values=BLANK)
    can_skip = ((ext != BLANK) & (ext != ext_m2)).astype(np.float32)

    m2f = np.zeros((B, SW), np.float32)
    m2f[:, :S] = can_skip
    m2b = np.zeros((B, SW), np.float32)
    for u in range(2, S):
        m2b[:, u] = can_skip[:, S - 1 - u + 2]

    mif = np.zeros((B, SW), np.float32)
    mif[:, 0] = 1.0
    mif[:, 1] = 1.0
    mib = np.zeros((B, SW), np.float32)
    mib[np.arange(B), S - 1 - 2 * lbl_len] = 1.0
    mib[np.arange(B), S - 1 - (2 * lbl_len - 1)] = 1.0

    idxf = np.full((B, NIDX), C, np.int16)          # pad -> zeroed column C
    idxf[:, :S] = ext.astype(np.int16)
    idxb = np.full((B, NIDX), C, np.int16)
    idxb[:, :S] = ext[:, ::-1].astype(np.int16)
    return m2f, m2b, mif, mib, idxf, idxb


def _wrap_idx(vals):
    """(144,) -> (128, 9) int16 in ap_gather's wrapped layout, replicated per
    16-partition gpsimd core: idx element j lives at [j % 16, j // 16]."""
    w = np.zeros((16, NIDX // 16), np.int16)
    for j in range(NIDX):
        w[j % 16, j // 16] = vals[j]
    return np.tile(w, (8, 1))


def _build(num_cores=NCORES, t_full=T, bn=BN):
    """Build and schedule the Bass module once. Returns (nc, names)."""
    import concourse.bacc as bacc
    import concourse.bass as bass
    import concourse.mybir as mybir
    import concourse.tile as tile
    from contextlib import ExitStack
    from concourse.vector_clock import ScopedClock

    # this walrus build allows a single sem wait per Drain: split the
    # TileContext end-drain's waits across a chain of drains.
    def _patched_drain_and_barrier(self, tick_clock, wait_clock):
        nc = self.nc
        drain_inst = nc.sync.drain()
        wait_clock.add_sem_waits(
            drain_inst.ins, ScopedClock({None: tick_clock.global_clock})
        )
        si = drain_inst.ins.sync_info
        waits = list(si.on_wait) if si and si.on_wait else []
        if len(waits) > 1:
            si.on_wait = waits[:1]
            for w in waits[1:]:
                extra = nc.sync.drain()
                esi = extra.ins.sync_info
                if esi is None:
                    extra.ins.sync_info = mybir.SyncInfo(on_wait=[w], on_update=[])
                else:
                    esi.on_wait = (esi.on_wait or []) + [w]
        nc.all_engine_barrier()
        assert self.sems is not None
        popped = nc._tile_sem_poison_stack.pop()
        assert popped is self._sem_poison
        nc.clear_and_free_semaphores(list(self.sems.allocated().values()))
        nc.all_engine_barrier()

    tile.TileContext._drain_and_barrier = _patched_drain_and_barrier

    f32 = mybir.dt.float32
    bf16 = mybir.dt.bfloat16
    i16 = mybir.dt.int16
    AX = mybir.AxisListType.X
    COPY = mybir.ActivationFunctionType.Copy

    th = t_full // 2
    nblk = t_full // 128          # t-blocks of 128
    chk = 64                      # tau-chunk size
    nchk = th // chk
    nresc = (th - 1) // RESC

    nc = bacc.Bacc("TRN2", target_bir_lowering=False, debug=False,
                   num_devices=num_cores)
    ypred = nc.dram_tensor("ypred", [bn, t_full, C], f32, kind="ExternalInput")
    m2_in = nc.dram_tensor("m2", [128, SW], bf16, kind="ExternalInput")
    mi_in = nc.dram_tensor("minit", [128, SW], bf16, kind="ExternalInput")
    idx_in = nc.dram_tensor("idx", [128, bn * 2 * (NIDX // 16)], i16,
                            kind="ExternalInput")
    h_out = nc.dram_tensor("hist", [128, max(nresc, 1)], f32, kind="ExternalOutput")
    a_out = nc.dram_tensor("afin", [128, SW + 2], bf16, kind="ExternalOutput")
    g_out = nc.dram_tensor("gfin", [128, SW], bf16, kind="ExternalOutput")

    with tile.TileContext(nc) as tc, ExitStack() as ctx:
        const = ctx.enter_context(tc.tile_pool(name="const", bufs=1))
        dramp = ctx.enter_context(tc.tile_pool(name="edram", bufs=1, space="DRAM"))
        ypf_pool = ctx.enter_context(tc.tile_pool(name="ypf", bufs=3))
        ef_pool = ctx.enter_context(tc.tile_pool(name="ef", bufs=4))
        ebf_pool = ctx.enter_context(tc.tile_pool(name="ebf", bufs=2))
        ec_pool = ctx.enter_context(tc.tile_pool(name="ec", bufs=2))

        m2t = const.tile([128, SW], bf16, tag="m2t")
        mit = const.tile([128, SW], bf16, tag="mit")
        idxt = const.tile([128, bn * 2 * (NIDX // 16)], i16, tag="idxt")
        alpha = const.tile([128, SW + 2], bf16, tag="alpha")
        ut = const.tile([128, SW], bf16, tag="ut")
        vt = const.tile([128, SW], bf16, tag="vt")
        histt = const.tile([128, max(nresc, 1)], f32, tag="histt")
        sclt = const.tile([128, 1], f32, tag="sclt")

        nc.sync.dma_start(out=m2t[:, :], in_=m2_in.ap())
        nc.sync.dma_start(out=mit[:, :], in_=mi_in.ap())
        nc.sync.dma_start(out=idxt[:, :], in_=idx_in.ap())
        nc.vector.memset(histt[:, :], 0.0)
        nc.vector.memset(alpha[:, :], 0.0)

        # ---- phase A: gather emissions per 128-t block, write to DRAM ----
        # block order so that chunk 0 (fwd t 0..63 + bwd t 511..448) is ready first
        blk_order = []
        for i in range(nblk // 2):
            blk_order += [i, nblk - 1 - i]
        edram = {}
        ngrp = bn // 16
        for k in blk_order:
            ebf_k = ebf_pool.tile([128, bn * NIDX], bf16)
            dirn = 0 if k < nblk // 2 else 1
            for grp in range(ngrp):
                ypf = ypf_pool.tile([128, 16 * CW], f32)
                yp3 = ypf[:, :].rearrange("p (e c) -> p e c", c=CW)
                src = ypred.ap()[grp * 16:(grp + 1) * 16,
                                 k * 128:(k + 1) * 128, :]
                nc.sync.dma_start(out=yp3[:, :, 0:C],
                                  in_=src.rearrange("e t c -> t e c"))
                nc.vector.memset(yp3[:, :, C:CW], 0.0)
                for e in range(16):
                    ex = grp * 16 + e
                    ef = ef_pool.tile([128, NIDX], f32)
                    ioff = (ex * 2 + dirn) * (NIDX // 16)
                    nc.gpsimd.ap_gather(
                        ef[:, :], ypf[:, e * CW:(e + 1) * CW],
                        idxt[:, ioff:ioff + NIDX // 16],
                        channels=128, num_elems=CW, d=1, num_idxs=NIDX)
                    nc.scalar.activation(
                        ebf_k[:, ex * NIDX:(ex + 1) * NIDX], ef[:, :],
                        COPY, bias=float(G * EPS), scale=float(G))
            ed = dramp.tile([128, bn * SW], bf16, tag=f"ed{k}")
            edram[k] = ed
            src3 = ebf_k[:, :].rearrange("p (e s) -> p e s", s=NIDX)
            dst3 = ed[:, :].rearrange("p (e s) -> p e s", s=SW)
            half = bn // 2
            nc.sync.dma_start(out=dst3[:, 0:half, :],
                              in_=src3[:, 0:half, 0:SW])
            nc.scalar.dma_start(out=dst3[:, half:bn, :],
                                in_=src3[:, half:bn, 0:SW])

        # ---- phase B: reshuffle DRAM -> (example|dir partition, tau) chunks ----
        ec_tiles = []
        for j in range(nchk):
            ec = ec_pool.tile([128, chk * SW], bf16)
            ec3 = ec[:, :].rearrange("p (t s) -> p t s", s=SW)
            kf = j // 2
            tl0 = (j % 2) * chk
            kb = nblk - 1 - kf
            tb0 = 127 - (j % 2) * chk
            fsrc = edram[kf][:, :].rearrange("t (e s) -> t e s", s=SW)
            bsrc = edram[kb][:, :].rearrange("t (e s) -> t e s", s=SW)
            bslice = slice(tb0, None, -1) if tb0 - chk < 0 else slice(tb0, tb0 - chk, -1)
            nc.sync.dma_start(
                out=ec3[0:64, :, :],
                in_=fsrc[tl0:tl0 + chk, :, :].rearrange("t e s -> e t s"))
            nc.scalar.dma_start(
                out=ec3[64:128, :, :],
                in_=bsrc[bslice, :, :].rearrange("t e s -> e t s"))
            ec_tiles.append(ec)

        # ---- phase C: the DP ----
        nc.vector.tensor_mul(alpha[:, 2:2 + SW], ec_tiles[0][:, 0:SW], mit[:, :])
        nr = 0
        for tau in range(1, th):
            ec = ec_tiles[tau // chk]
            off = (tau % chk) * SW
            nc.vector.tensor_add(ut[:, :], alpha[:, 2:2 + SW], alpha[:, 1:1 + SW])
            nc.vector.tensor_mul(vt[:, :], alpha[:, 0:SW], m2t[:, :])
            nc.vector.tensor_add(ut[:, :], ut[:, :], vt[:, :])
            nc.vector.tensor_mul(alpha[:, 2:2 + SW], ut[:, :], ec[:, off:off + SW])
            if tau % RESC == 0 and nr < nresc:
                nc.vector.reduce_max(histt[:, nr:nr + 1], alpha[:, 2:2 + S], axis=AX)
                nc.vector.reciprocal(sclt[:, :], histt[:, nr:nr + 1])
                nc.vector.tensor_scalar(alpha[:, 2:2 + SW], alpha[:, 2:2 + SW],
                                        sclt[:, :], float(2.0 ** SETPOINT_LOG2),
                                        mybir.AluOpType.mult,
                                        mybir.AluOpType.mult)
                nr += 1

        # ---- final: gamma on bwd rows, dump states (host does f64 splice) ----
        nc.vector.tensor_add(ut[:, :], alpha[:, 2:2 + SW], alpha[:, 1:1 + SW])
        nc.vector.tensor_mul(vt[:, :], alpha[:, 0:SW], m2t[:, :])
        nc.vector.tensor_add(ut[:, :], ut[:, :], vt[:, :])
        nc.sync.dma_start(out=a_out.ap(), in_=alpha[:, :])
        nc.sync.dma_start(out=g_out.ap(), in_=ut[:, :])
        nc.sync.dma_start(out=h_out.ap(), in_=histt[:, :])

    nc.compile()
    return nc


def kernel(y_true, y_pred):
    global _BUILT
    from concourse.bass_utils import run_bass_kernel_spmd

    y_true = np.asarray(y_true)
    y_pred = np.ascontiguousarray(np.asarray(y_pred, dtype=np.float32))

    m2f, m2b, mif, mib, idxf, idxb = _host_metadata(y_true)

    if _BUILT is None:
        _BUILT = _build()
    nc = _BUILT

    import ml_dtypes
    bf = ml_dtypes.bfloat16
    in_maps = []
    for c in range(NCORES):
        sl = slice(c * BN, (c + 1) * BN)
        m2 = np.concatenate([m2f[sl], m2b[sl]], axis=0).astype(bf)
        mi = np.concatenate([mif[sl], mib[sl]], axis=0).astype(bf)
        idx = np.zeros((128, BN * 2 * (NIDX // 16)), np.int16)
        for e in range(BN):
            b = c * BN + e
            idx[:, (e * 2) * 9:(e * 2 + 1) * 9] = _wrap_idx(idxf[b])
            idx[:, (e * 2 + 1) * 9:(e * 2 + 2) * 9] = _wrap_idx(idxb[b])
        in_maps.append({
            "ypred": y_pred[sl],
            "m2": m2,
            "minit": mi,
            "idx": idx,
        })

    import os
    trace = os.environ.get("CTC_TRACE", "") == "1"
    res = run_bass_kernel_spmd(nc, in_maps, list(range(NCORES)), trace=trace)
    global _LAST_EXEC_NS, _LAST_RES
    _LAST_EXEC_NS = res.exec_time_ns
    _LAST_RES = res

    losses = np.zeros(B, np.float64)
    lng = np.log(np.float64(G))
    setlog = NRESC * SETPOINT_LOG2 * np.log(2.0)
    for c in range(NCORES):
        afin = res.results[c]["afin"].astype(np.float64)   # (128, SW+2)
        gfin = res.results[c]["gfin"].astype(np.float64)   # (128, SW)
        hist = res.results[c]["hist"].astype(np.float64)
        acc = np.log(np.maximum(hist[:, :NRESC], 1e-300)).sum(axis=1)
        af = afin[0:64, 2:2 + S]                 # alpha_{T/2-1}[s], s ascending
        gm = gfin[64:128, 0:S][:, ::-1]          # W_{T/2-1}[s], reversed u->s
        P = (af * gm).sum(axis=1)
        lnP = np.log(np.maximum(P, 1e-300))
        losses[c * BN:(c + 1) * BN] = -(
            lnP + acc[:64] + acc[64:128] - 2 * setlog - T * lng)
    return np.float32(losses.mean())
